# revision 1
# baseline (speedup 1.0000x reference)
"""Decoder-only transformer forward on 8 trn2 NeuronCores.

Sharding (SPMD, zero AllReduce):
  - residual stream token-sharded: core c owns flat tokens [256c, 256c+256)
  - attention head-sharded: core c owns heads (2c, 2c+1) over all 2048 tokens
  - Wo / FFN token-sharded (full weights streamed per core)
  - LM head vocab-sharded: core c owns padded-vocab cols [4096c, 4096c+4096)
  Collectives per layer: AllGather(h = LN1 out, 1MB/rank), AllToAll(ctx, 1MB),
  plus one final AllGather of the LN'd trunk output for the head.

Layouts: activations feature-major ("T": [d partitions, tokens free]) so every
matmul consumes natural operands; scores computed transposed ([k, q]) so the
pad-key bias is a per-partition scalar and V-hat's appended ones-row yields the
softmax normalizer from the same accumulation.

Matmuls in float32r (TF32-like, full PE rate at N>=256); LN stats via PE
ones-matmul partition reductions.
"""

import math
import os

import numpy as np

PROBE = bool(os.environ.get("BASS_PROBE"))

B, S, D, H, L, F, V = 2, 1024, 1024, 16, 6, 4096, 32000
NCORES = 8
T = B * S                 # 2048 flat tokens
TPC = T // NCORES         # 256 tokens per core
VPAD = 32768
VPC = VPAD // NCORES      # 4096 vocab cols per core
DK = D // H               # 64
HPC = H // NCORES         # 2 heads per core
NEG = -1e9
EPS = 1e-5

_CACHE = {}


def _build():
    import concourse.mybir as mybir
    import concourse.tile as tile
    from concourse import bacc
    from concourse.masks import make_identity

    dtr = mybir.dt.float32r
    dtf = mybir.dt.float32
    AF = mybir.ActivationFunctionType
    OP = mybir.AluOpType

    nc = bacc.Bacc(
        "TRN2",
        target_bir_lowering=False,
        debug=False,
        enable_asserts=False,
        num_devices=NCORES,
    )
    RG = [list(range(NCORES))]

    # ---- I/O ----
    x0t_i = nc.dram_tensor("x0t", [D, TPC], dtr, kind="ExternalInput")
    wq_i = nc.dram_tensor("wq", [L, D, 128], dtr, kind="ExternalInput")
    wk_i = nc.dram_tensor("wk", [L, D, 128], dtr, kind="ExternalInput")
    wv_i = nc.dram_tensor("wv", [L, D, 128], dtr, kind="ExternalInput")
    wo_i = nc.dram_tensor("wo", [L, D, D], dtr, kind="ExternalInput")
    w1_i = nc.dram_tensor("w1", [L, D, F], dtr, kind="ExternalInput")
    w2_i = nc.dram_tensor("w2", [L, F, D], dtr, kind="ExternalInput")
    wout_i = nc.dram_tensor("wout", [D, VPC], dtr, kind="ExternalInput")
    # per-feature params in T layout ([128, n_chunks] per layer)
    g1_i = nc.dram_tensor("g1", [L, 128, 8], dtf, kind="ExternalInput")
    be1_i = nc.dram_tensor("be1", [L, 128, 8], dtf, kind="ExternalInput")
    g2_i = nc.dram_tensor("g2", [L, 128, 8], dtf, kind="ExternalInput")
    be2_i = nc.dram_tensor("be2", [L, 128, 8], dtf, kind="ExternalInput")
    bo_i = nc.dram_tensor("bo", [L, 128, 8], dtf, kind="ExternalInput")
    b1_i = nc.dram_tensor("b1", [L, 128, 32], dtf, kind="ExternalInput")
    b2_i = nc.dram_tensor("b2", [L, 128, 8], dtf, kind="ExternalInput")
    gf_i = nc.dram_tensor("gf", [128, 8], dtf, kind="ExternalInput")
    bf_i = nc.dram_tensor("bf", [128, 8], dtf, kind="ExternalInput")
    bout_i = nc.dram_tensor("bout", [128, 32], dtf, kind="ExternalInput")
    causal_i = nc.dram_tensor("causal", [128, 128], dtf, kind="ExternalInput")
    padb_i = nc.dram_tensor("padb", [2, 8, 128], dtf, kind="ExternalInput")
    out_o = nc.dram_tensor("logits_t", [VPC, T], dtf, kind="ExternalOutput")

    with tile.TileContext(nc) as tc:
        _body(
            nc, tc, mybir, dtr, dtf, AF, OP, RG, make_identity,
            x0t_i, wq_i, wk_i, wv_i, wo_i, w1_i, w2_i, wout_i,
            g1_i, be1_i, g2_i, be2_i, bo_i, b1_i, b2_i, gf_i, bf_i, bout_i,
            causal_i, padb_i, out_o,
        )
    nc.compile()
    return nc


def _body(nc, tc, mybir, dtr, dtf, AF, OP, RG, make_identity,
          x0t_i, wq_i, wk_i, wv_i, wo_i, w1_i, w2_i, wout_i,
          g1_i, be1_i, g2_i, be2_i, bo_i, b1_i, b2_i, gf_i, bf_i, bout_i,
          causal_i, padb_i, out_o):
    import contextlib
    dtb = mybir.dt.bfloat16
    ctx = contextlib.ExitStack()
    with ctx:
        const = ctx.enter_context(tc.tile_pool(name="const", bufs=1))
        lnp = ctx.enter_context(tc.tile_pool(name="lnp", bufs=1))
        resid = ctx.enter_context(tc.tile_pool(name="resid", bufs=1))
        act = ctx.enter_context(tc.tile_pool(name="act", bufs=1))
        act2 = ctx.enter_context(tc.tile_pool(name="act2", bufs=2))
        hfp = ctx.enter_context(tc.tile_pool(name="hfp", bufs=9))
        expp = ctx.enter_context(tc.tile_pool(name="expp", bufs=3))
        ctxp = ctx.enter_context(tc.tile_pool(name="ctxp", bufs=9))
        wp1 = ctx.enter_context(tc.tile_pool(name="wp1", bufs=1))
        wp2 = ctx.enter_context(tc.tile_pool(name="wp2", bufs=2))
        stat = ctx.enter_context(tc.tile_pool(name="stat", bufs=1))
        ps = ctx.enter_context(tc.tile_pool(name="ps", bufs=4, space="PSUM"))
        dram = ctx.enter_context(tc.tile_pool(name="dram", bufs=2, space="DRAM"))

        # ---- constants (staged through one f32 scratch tag) ----
        stage = const.tile([128, 128], dtf, tag="stage")
        make_identity(nc, stage[:])
        ident = const.tile([128, 128], dtr, tag="ident")
        nc.scalar.copy(out=ident[:], in_=stage[:])
        nc.vector.memset(stage[:, 0:1], 1.0)
        ones_col = const.tile([128, 1], dtr, tag="ones_col")
        nc.scalar.copy(out=ones_col[:], in_=stage[:, 0:1])
        onesb_col = const.tile([128, 1], dtb, tag="onesb_col")
        nc.scalar.copy(out=onesb_col[:], in_=stage[:, 0:1])
        nc.vector.memset(stage[0:1, :], 1.0)
        ones_row = const.tile([1, 128], dtr, tag="ones_row")
        nc.scalar.copy(out=ones_row[:], in_=stage[0:1, :])
        eps_t = const.tile([1, 1], dtf, tag="eps_t")
        nc.vector.memset(eps_t[:], 1e-5)

        causal = const.tile([128, 128], dtf, tag="causal")
        nc.sync.dma_start(out=causal[:], in_=causal_i[:, :])
        padb = const.tile([128, 16], dtf, tag="padb")
        nc.sync.dma_start(out=padb[:], in_=padb_i.ap().rearrange("b k p -> p (b k)"))

        # ---- persistent residual: feature-major, chunk i = cols [256i, 256i+256) ----
        xT = resid.tile([128, 2048], dtr, tag="xT")
        nc.sync.dma_start(
            out=xT[:].rearrange("p (i t) -> p i t", i=8),
            in_=x0t_i.ap().rearrange("(i p) t -> p i t", p=128),
        )

        def cs(ap, i, w=256):
            return ap[:, i * w:(i + 1) * w]

        def layer_norm(src_tile, g_t, b_t):
            """src [128, 2048] f32r -> normalized [128, 2048] f32r."""
            p_s = ps.tile([128, 1024], dtf, tag="bank")
            for i in range(8):
                sq = act2.tile([128, 256], dtr, tag="ln_sq")
                nc.vector.tensor_mul(out=sq[:], in0=cs(src_tile[:], i),
                                     in1=cs(src_tile[:], i))
                nc.tensor.matmul(p_s[0:1, 0:256], ones_col[:],
                                 cs(src_tile[:], i),
                                 start=(i == 0), stop=(i == 7))
                # start=True clears has_written for the whole bank, so the
                # sq group must NOT restart it: first write lands on cleared
                # bits (overwrite), later writes accumulate.
                nc.tensor.matmul(p_s[0:1, 256:512], ones_col[:], sq[:],
                                 start=False, stop=(i == 7))
            mean_r = stat.tile([1, 256], dtr, tag="mean_r")
            nc.scalar.activation(out=mean_r[:], in_=p_s[0:1, 0:256],
                                 func=AF.Copy, scale=1.0 / 1024.0)
            sc = stat.tile([1, 1024], dtf, tag="sc")  # ex2|msq|var|std
            nc.scalar.activation(out=sc[0:1, 0:256], in_=p_s[0:1, 256:512],
                                 func=AF.Copy, scale=1.0 / 1024.0)
            nc.vector.tensor_mul(out=sc[0:1, 256:512], in0=mean_r[:],
                                 in1=mean_r[:])
            nc.vector.tensor_sub(out=sc[0:1, 512:768], in0=sc[0:1, 0:256],
                                 in1=sc[0:1, 256:512])
            nc.scalar.activation(out=sc[0:1, 768:1024], in_=sc[0:1, 512:768],
                                 func=AF.Sqrt, bias=eps_t[:])
            rstd_r = stat.tile([1, 256], dtr, tag="rstd_r")
            with nc.allow_low_precision(reason="f32r rounding for matmul"):
                nc.vector.reciprocal(out=rstd_r[:], in_=sc[0:1, 768:1024])
            p_b = ps.tile([128, 1024], dtf, tag="bank")
            nc.tensor.matmul(p_b[:, 0:256], ones_row[:], mean_r[:],
                             start=True, stop=True)
            nc.tensor.matmul(p_b[:, 256:512], ones_row[:], rstd_r[:],
                             start=True, stop=True)
            out_t = act.tile([128, 2048], dtr, tag="ln_out")
            for i in range(8):
                tmp = act2.tile([128, 256], dtr, tag="ln_tmp")
                nc.vector.tensor_sub(out=tmp[:], in0=cs(src_tile[:], i),
                                     in1=p_b[:, 0:256])
                nc.vector.tensor_mul(out=tmp[:], in0=tmp[:],
                                     in1=p_b[:, 256:512])
                nc.vector.tensor_scalar(
                    out=cs(out_t[:], i), in0=tmp[:],
                    scalar1=g_t[:, i:i + 1], scalar2=b_t[:, i:i + 1],
                    op0=OP.mult, op1=OP.add)
            return out_t

        def ln_param(dram_t, tag, idx=None):
            shp = [128, dram_t.shape[-1]]
            t = lnp.tile(shp, dtf, tag=tag)
            nc.sync.dma_start(out=t[:], in_=dram_t[idx] if idx is not None
                              else dram_t[:, :])
            return t

        def probe_dump(slot, tile_ap):
            nc.sync.dma_start(
                out=out_o[128 * slot:128 * (slot + 1), :], in_=tile_ap)

        for l in range(1 if PROBE else L):
            # ---- LN1 -> h, AllGather h ----
            hT = layer_norm(xT, ln_param(g1_i, "g1t", l),
                            ln_param(be1_i, "be1t", l))
            ag_h_in = dram.tile([D, TPC], dtr, tag="ag_h_in")
            nc.sync.dma_start(
                out=ag_h_in[:].rearrange("(c p) t -> p c t", p=128),
                in_=hT[:].rearrange("p (c t) -> p c t", c=8),
            )
            ag_h_out = dram.tile([NCORES * D, TPC], dtr, tag="ag_h_out")
            nc.gpsimd.collective_compute(
                "AllGather", mybir.AluOpType.bypass, replica_groups=RG,
                ins=[ag_h_in.opt()], outs=[ag_h_out.opt()])
            agv = ag_h_out[:].rearrange("(r d) t -> d r t", r=NCORES)

            # ---- QKV (own 2 heads, all tokens), half the tokens at a time ----
            wqt = wp1.tile([128, 8, 128], dtr, tag="wqt")
            nc.sync.dma_start(out=wqt[:],
                              in_=wq_i[l].rearrange("(c p) m -> p c m", p=128))
            wkt = wp1.tile([128, 8, 128], dtr, tag="wkt")
            nc.sync.dma_start(out=wkt[:],
                              in_=wk_i[l].rearrange("(c p) m -> p c m", p=128))
            wvt = wp1.tile([128, 8, 128], dtr, tag="wvt")
            nc.sync.dma_start(out=wvt[:],
                              in_=wv_i[l].rearrange("(c p) m -> p c m", p=128))
            qT = act.tile([128, 2048], dtr, tag="qT")
            kT = act.tile([128, 2048], dtr, tag="kT")
            vN = act.tile([128, 16, 130], dtb, tag="vN")
            for tb in range(16):
                nc.vector.tensor_copy(out=vN[:, tb, 64:65], in_=onesb_col[:])
                nc.vector.tensor_copy(out=vN[:, tb, 129:130], in_=onesb_col[:])
            for half in range(2):
                hf = []
                for c in range(8):
                    h_c = hfp.tile([128, 1024], dtr, tag="hfull")
                    nc.sync.dma_start(
                        out=h_c[:].rearrange("p (r t) -> p r t", r=4),
                        in_=agv[128 * c:128 * (c + 1), 4 * half:4 * (half + 1)])
                    hf.append(h_c)
                for name, wt, dst in (("q", wqt, qT), ("k", wkt, kT)):
                    p_p = ps.tile([128, 1024], dtf, tag="bank")
                    for c in range(8):
                        for j in range(2):
                            nc.tensor.matmul(
                                p_p[:, 512 * j:512 * (j + 1)], wt[:, c, :],
                                hf[c][:, 512 * j:512 * (j + 1)],
                                start=(c == 0), stop=(c == 7))
                    if name == "q":
                        nc.vector.tensor_copy(
                            out=dst[:, 1024 * half:1024 * (half + 1)],
                            in_=p_p[:])
                    else:
                        nc.scalar.copy(
                            out=dst[:, 1024 * half:1024 * (half + 1)],
                            in_=p_p[:])
                # V: project then transpose into token-major vN
                p_v = ps.tile([128, 1024], dtf, tag="bank")
                for c in range(8):
                    for j in range(2):
                        nc.tensor.matmul(
                            p_v[:, 512 * j:512 * (j + 1)], wvt[:, c, :],
                            hf[c][:, 512 * j:512 * (j + 1)],
                            start=(c == 0), stop=(c == 7))
                vT = act2.tile([128, 1024], dtr, tag="vT")
                nc.vector.tensor_copy(out=vT[:], in_=p_v[:])
                for tg in range(2):
                    p_vt = ps.tile([128, 1024], dtr, tag="bank")
                    for u in range(4):
                        nc.tensor.transpose(
                            p_vt[:, 128 * u:128 * (u + 1)],
                            vT[:, 128 * (4 * tg + u):128 * (4 * tg + u + 1)],
                            ident[:])
                    for u in range(4):
                        tb = 8 * half + 4 * tg + u
                        nc.vector.tensor_copy(
                            out=vN[:, tb, 0:64],
                            in_=p_vt[:, 128 * u:128 * u + 64])
                        nc.scalar.copy(
                            out=vN[:, tb, 65:129],
                            in_=p_vt[:, 128 * u + 64:128 * u + 128])

            if PROBE:
                probe_dump(0, hT[:].bitcast(dtf))
                probe_dump(1, qT[:].bitcast(dtf))
                probe_dump(2, kT[:].bitcast(dtf))

            # ---- attention per (head, batch); scores transposed [k, q] ----
            ctx_sb = act.tile([128, 2048], dtr, tag="ctx_sb")
            for hh in range(2):
                for b in range(B):
                    qs = qT[64 * hh:64 * (hh + 1), 1024 * b:1024 * (b + 1)]
                    ks = kT[64 * hh:64 * (hh + 1), 1024 * b:1024 * (b + 1)]
                    p_u = ps.tile([128, 1024], dtf, tag="bank")
                    for kb in range(8):
                        live = 1024 - 128 * kb
                        p_sc = ps.tile([128, 1024], dtf, tag="bank")
                        off = 0
                        while off < live:
                            w = min(512, live - off)
                            nc.tensor.matmul(
                                p_sc[:, off:off + w],
                                ks[:, 128 * kb:128 * (kb + 1)],
                                qs[:, 128 * kb + off:128 * kb + off + w],
                                start=True, stop=True)
                            off += w
                        nc.vector.tensor_add(out=p_sc[:, 0:128],
                                             in0=p_sc[:, 0:128], in1=causal[:])
                        nc.vector.tensor_scalar(
                            out=p_sc[:, 0:live], in0=p_sc[:, 0:live],
                            scalar1=padb[:, 8 * b + kb:8 * b + kb + 1],
                            scalar2=None, op0=OP.add)
                        es = expp.tile([128, 1024], dtb, tag="expS")
                        nc.scalar.activation(out=es[:, 0:live],
                                             in_=p_sc[:, 0:live], func=AF.Exp)
                        # U = [1 | V].T @ expS accumulated over k blocks
                        vsl = vN[:, 8 * b + kb, 65 * hh:65 * (hh + 1)]
                        off = 0
                        while off < live:
                            w = min(512, live - off)
                            nc.tensor.matmul(
                                p_u[0:65, 128 * kb + off:128 * kb + off + w],
                                vsl, es[:, off:off + w],
                                start=(kb == 0), stop=(kb == 7))
                            off += w
                    # rows: 0:64 = unnormalized ctx, 64 = sum(exp)
                    rc = stat.tile([1, 1024], dtf, tag="rc")
                    nc.vector.reciprocal(out=rc[:], in_=p_u[64:65, :])
                    rbb = stat.tile([64, 1024], dtf, tag="rbb")
                    nc.gpsimd.partition_broadcast(rbb[:], rc[:])
                    nc.vector.tensor_mul(
                        out=ctx_sb[64 * hh:64 * (hh + 1),
                                   1024 * b:1024 * (b + 1)],
                        in0=p_u[0:64, :], in1=rbb[:])

            # ---- ctx AllToAll: shard j = my heads x rank-j tokens ----
            a2a_in = dram.tile([NCORES * 128, TPC], dtr, tag="a2a_in")
            for j in range(NCORES):
                nc.sync.dma_start(out=a2a_in[128 * j:128 * (j + 1), :],
                                  in_=cs(ctx_sb[:], j))
            a2a_out = dram.tile([NCORES * 128, TPC], dtr, tag="a2a_out")
            nc.gpsimd.collective_compute(
                "AllToAll", mybir.AluOpType.bypass, replica_groups=RG,
                ins=[a2a_in.opt()], outs=[a2a_out.opt()])
            ctxf = []
            for c in range(8):
                cf = ctxp.tile([128, 256], dtr, tag="ctxf")
                nc.sync.dma_start(out=cf[:],
                                  in_=a2a_out[128 * c:128 * (c + 1), :])
                ctxf.append(cf)

            # ---- Wo + bias + residual (own tokens) ----
            bot = ln_param(bo_i, "bot", l)
            for m in range(8):
                wot = wp2.tile([128, 8, 128], dtr, tag="wot")
                nc.sync.dma_start(
                    out=wot[:],
                    in_=wo_i[l][:, 128 * m:128 * (m + 1)].rearrange(
                        "(c p) n -> p c n", p=128))
                p_y = ps.tile([128, 1024], dtf, tag="bank")
                for c in range(8):
                    nc.tensor.matmul(p_y[:, 0:256], wot[:, c, :], ctxf[c][:],
                                     start=(c == 0), stop=(c == 7))
                tmp = act2.tile([128, 256], dtr, tag="res_tmp")
                nc.vector.tensor_scalar(
                    out=tmp[:], in0=p_y[:, 0:256], scalar1=bot[:, m:m + 1],
                    scalar2=None, op0=OP.add)
                nc.vector.tensor_add(out=cs(xT[:], m), in0=cs(xT[:], m),
                                     in1=tmp[:])

            if PROBE:
                probe_dump(3, ctx_sb[:].bitcast(dtf))
                probe_dump(4, xT[:].bitcast(dtf))

            # ---- LN2 + FFN ----
            h2T = layer_norm(xT, ln_param(g2_i, "g2t", l),
                             ln_param(be2_i, "be2t", l))
            b1t = ln_param(b1_i, "b1t", l)
            # y2 accumulates in PSUM across all 4 F-groups
            p_y2a = ps.tile([128, 1024], dtf, tag="bank")
            p_y2b = ps.tile([128, 1024], dtf, tag="bank")
            for g in range(4):
                p_u1 = ps.tile([128, 1024], dtf, tag="bank")
                p_u2 = ps.tile([128, 1024], dtf, tag="bank")
                for c in range(8):
                    w1t = wp2.tile([128, 1024], dtr, tag="w1t")
                    nc.sync.dma_start(
                        out=w1t[:],
                        in_=w1_i[l, 128 * c:128 * (c + 1),
                                 1024 * g:1024 * (g + 1)])
                    for fl in range(8):
                        pu = p_u1 if fl < 4 else p_u2
                        nc.tensor.matmul(
                            pu[:, 256 * (fl % 4):256 * (fl % 4 + 1)],
                            w1t[:, 128 * fl:128 * (fl + 1)], cs(h2T[:], c),
                            start=(c == 0 and fl % 2 == 0), stop=(c == 7))
                guT = act2.tile([128, 2048], dtr, tag="guT")
                for fl in range(8):
                    pu = p_u1 if fl < 4 else p_u2
                    fc = 8 * g + fl
                    nc.scalar.activation(
                        out=cs(guT[:], fl), in_=pu[:, 256 * (fl % 4):256 * (fl % 4 + 1)],
                        func=AF.Gelu, bias=b1t[:, fc:fc + 1])
                for fl in range(8):
                    fc = 8 * g + fl
                    w2t = wp2.tile([128, 1024], dtr, tag="w2t")
                    nc.sync.dma_start(
                        out=w2t[:], in_=w2_i[l, 128 * fc:128 * (fc + 1), :])
                    for mq in range(8):
                        py = p_y2a if mq < 4 else p_y2b
                        nc.tensor.matmul(
                            py[:, 256 * (mq % 4):256 * (mq % 4 + 1)],
                            w2t[:, 128 * mq:128 * (mq + 1)], cs(guT[:], fl),
                            start=(g == 0 and fl == 0 and mq % 2 == 0),
                            stop=(g == 3 and fl == 7))
            b2t = ln_param(b2_i, "b2t", l)
            for m in range(8):
                py = p_y2a if m < 4 else p_y2b
                tmp2 = act2.tile([128, 256], dtr, tag="res_tmp")
                nc.vector.tensor_scalar(
                    out=tmp2[:], in0=py[:, 256 * (m % 4):256 * (m % 4 + 1)],
                    scalar1=b2t[:, m:m + 1], scalar2=None, op0=OP.add)
                nc.vector.tensor_add(out=cs(xT[:], m), in0=cs(xT[:], m),
                                     in1=tmp2[:])

        if PROBE:
            probe_dump(5, xT[:].bitcast(dtf))
            return

        # ---- final LN + AllGather ----
        xfT = layer_norm(xT, ln_param(gf_i, "gft"), ln_param(bf_i, "bft"))
        ag_f_in = dram.tile([D, TPC], dtr, tag="ag_h_in")
        nc.sync.dma_start(
            out=ag_f_in[:].rearrange("(c p) t -> p c t", p=128),
            in_=xfT[:].rearrange("p (c t) -> p c t", c=8),
        )
        ag_f_out = dram.tile([NCORES * D, TPC], dtr, tag="ag_h_out")
        nc.gpsimd.collective_compute(
            "AllGather", mybir.AluOpType.bypass, replica_groups=RG,
            ins=[ag_f_in.opt()], outs=[ag_f_out.opt()])
        agf = ag_f_out[:].rearrange("(r d) t -> d r t", r=NCORES)

        # ---- LM head (own vocab slice), half the tokens at a time ----
        boutt = ln_param(bout_i, "boutt")
        for half in range(2):
            xff = []
            for c in range(8):
                xf = hfp.tile([128, 1024], dtr, tag="hfull")
                nc.sync.dma_start(
                    out=xf[:].rearrange("p (r t) -> p r t", r=4),
                    in_=agf[128 * c:128 * (c + 1), 4 * half:4 * (half + 1)])
                xff.append(xf)
            for vm in range(32):
                wvh = wp2.tile([128, 8, 128], dtr, tag="wvh")
                nc.sync.dma_start(
                    out=wvh[:],
                    in_=wout_i.ap()[:, 128 * vm:128 * (vm + 1)].rearrange(
                        "(c p) m -> p c m", p=128))
                p_o = ps.tile([128, 1024], dtf, tag="bank")
                for c in range(8):
                    for j in range(2):
                        nc.tensor.matmul(
                            p_o[:, 512 * j:512 * (j + 1)], wvh[:, c, :],
                            xff[c][:, 512 * j:512 * (j + 1)],
                            start=(c == 0), stop=(c == 7))
                osb = act2.tile([128, 1024], dtf, tag="osb")
                nc.vector.tensor_scalar(
                    out=osb[:], in0=p_o[:], scalar1=boutt[:, vm:vm + 1],
                    scalar2=None, op0=OP.add)
                nc.sync.dma_start(
                    out=out_o[128 * vm:128 * (vm + 1),
                              1024 * half:1024 * (half + 1)],
                    in_=osb[:])


def _host_inputs(inputs):
    tokens = np.asarray(inputs["tokens"])
    emb = np.asarray(inputs["emb"], dtype=np.float32)
    pe = np.asarray(inputs["pe"], dtype=np.float32)
    Wq = np.asarray(inputs["Wq"], dtype=np.float32)
    Wk = np.asarray(inputs["Wk"], dtype=np.float32)
    Wv = np.asarray(inputs["Wv"], dtype=np.float32)
    Wo = np.asarray(inputs["Wo"], dtype=np.float32)
    bo = np.asarray(inputs["bo"], dtype=np.float32)
    g1 = np.asarray(inputs["g1"], dtype=np.float32)
    be1 = np.asarray(inputs["be1"], dtype=np.float32)
    g2 = np.asarray(inputs["g2"], dtype=np.float32)
    be2 = np.asarray(inputs["be2"], dtype=np.float32)
    W1 = np.asarray(inputs["W1"], dtype=np.float32)
    b1 = np.asarray(inputs["b1"], dtype=np.float32)
    W2 = np.asarray(inputs["W2"], dtype=np.float32)
    b2 = np.asarray(inputs["b2"], dtype=np.float32)
    gf = np.asarray(inputs["gf"], dtype=np.float32)
    bf = np.asarray(inputs["bf"], dtype=np.float32)
    Wout = np.asarray(inputs["Wout"], dtype=np.float32)
    bout = np.asarray(inputs["bout"], dtype=np.float32)

    x0 = emb[tokens] * math.sqrt(float(D)) + pe[:S][None]   # (B, S, D)
    xflat = np.ascontiguousarray(x0.reshape(T, D))

    padb = np.where(tokens == 0, np.float32(NEG), np.float32(0.0))
    padb = np.ascontiguousarray(padb.reshape(2, 8, 128).astype(np.float32))
    r = np.arange(128)
    causal = np.where(r[:, None] > r[None, :], np.float32(NEG),
                      np.float32(0.0)).astype(np.float32)

    def tchunks(a, n):   # [L, D'] -> [L, 128, n] feature-major chunks
        return np.ascontiguousarray(
            a.reshape(L, n, 128).transpose(0, 2, 1).astype(np.float32))

    g1t, be1t = tchunks(g1, 8), tchunks(be1, 8)
    g2t, be2t = tchunks(g2, 8), tchunks(be2, 8)
    bot, b2t = tchunks(bo, 8), tchunks(b2, 8)
    b1t = tchunks(b1, 32)
    gft = np.ascontiguousarray(gf.reshape(8, 128).T.astype(np.float32))
    bft = np.ascontiguousarray(bf.reshape(8, 128).T.astype(np.float32))

    wout_p = np.zeros((D, VPAD), dtype=np.float32)
    wout_p[:, :V] = Wout
    bout_p = np.zeros((VPAD,), dtype=np.float32)
    bout_p[:V] = bout

    wq_s = (Wq / math.sqrt(float(DK))).astype(np.float32)

    shared = dict(
        wo=np.ascontiguousarray(Wo), w1=np.ascontiguousarray(W1),
        w2=np.ascontiguousarray(W2),
        g1=g1t, be1=be1t, g2=g2t, be2=be2t, bo=bot, b1=b1t, b2=b2t,
        gf=gft, bf=bft, causal=causal, padb=padb,
    )
    in_maps = []
    for c in range(NCORES):
        hc = slice(128 * c, 128 * (c + 1))
        m = dict(shared)
        m["x0t"] = np.ascontiguousarray(
            xflat[TPC * c:TPC * (c + 1)].T)
        m["wq"] = np.ascontiguousarray(wq_s[:, :, hc])
        m["wk"] = np.ascontiguousarray(Wk[:, :, hc])
        m["wv"] = np.ascontiguousarray(Wv[:, :, hc])
        m["wout"] = np.ascontiguousarray(wout_p[:, VPC * c:VPC * (c + 1)])
        m["bout"] = np.ascontiguousarray(
            bout_p[VPC * c:VPC * (c + 1)].reshape(32, 128).T)
        in_maps.append(m)
    return in_maps


def kernel(**inputs):
    from concourse import bass_utils

    if "nc" not in _CACHE:
        _CACHE["nc"] = _build()
    nc = _CACHE["nc"]
    in_maps = _host_inputs(inputs)
    res = bass_utils.run_bass_kernel_spmd(
        nc, in_maps, core_ids=list(range(NCORES)))
    full = np.empty((T, V), dtype=np.float32)
    for c in range(NCORES):
        lt = res.results[c]["logits_t"]          # [VPC, T]
        lo = VPC * c
        w = min(VPC, V - lo)
        if w > 0:
            full[:, lo:lo + w] = lt[:w, :].T
    return full.reshape(B, S, V)


if __name__ == "__main__":
    import sys
    sys.path.insert(0, "/root/problem")
    import reference
    inp = reference.setup_inputs()
    out = kernel(**{k: np.asarray(v) for k, v in inp.items()})
    print("kernel output", out.shape, out.dtype)



# revision 11
# speedup vs baseline: 1.0832x; 1.0832x over previous
"""Decoder-only transformer forward on 8 trn2 NeuronCores.

Sharding (SPMD, two small AllToAlls per layer, no AllGather):
  - residual stream token-sharded: core c owns flat tokens [256c, 256c+256)
  - QKV computed locally on own tokens with FULL Wq/Wk/Wv (bf16), then one
    AllToAll redistributes q,k,v head-sharded: core c gets heads (2c, 2c+1)
    for all 2048 tokens (1.5 MB bf16 per rank)
  - attention head-sharded; ctx AllToAll'd back token-sharded (0.5 MB bf16)
  - Wo / FFN token-sharded (full bf16 weights streamed per core)
  - LM head token-sharded: full Wout (bf16) streamed per core, logits out
    bf16 — no final collective.

Layouts: activations feature-major ("T": [d partitions, tokens free]); V is
produced token-major directly by swapping matmul operands (no PE
transposes); scores computed transposed ([k, q]) so the pad-key bias is a
per-partition scalar and V-hat's appended ones-row yields the softmax
normalizer from the same accumulation.

Matmul weights/activations bf16 (full PE rate, half DMA + A2A wire);
PSUM accumulation fp32; residual stream fp32r; LN stats via PE
ones-matmul partition reductions.
"""

import math
import os

import numpy as np
import ml_dtypes

PROBE = bool(os.environ.get("BASS_PROBE"))

B, S, D, H, L, F, V = 2, 1024, 1024, 16, 6, 4096, 32000
NCORES = 8
T = B * S                 # 2048 flat tokens
TPC = T // NCORES         # 256 tokens per core
VPAD = 32768
DK = D // H               # 64
NEG = -1e9
EPS = 1e-5

_CACHE = {}


def _build():
    import concourse.mybir as mybir
    import concourse.tile as tile
    from concourse import bacc
    from concourse.masks import make_identity

    dtr = mybir.dt.float32r
    dtf = mybir.dt.float32
    dtb = mybir.dt.bfloat16

    nc = bacc.Bacc(
        "TRN2",
        target_bir_lowering=False,
        debug=False,
        enable_asserts=False,
        num_devices=NCORES,
    )
    RG = [list(range(NCORES))]

    # ---- I/O ----
    x0t_i = nc.dram_tensor("x0t", [D, TPC], dtr, kind="ExternalInput")
    wq_i = nc.dram_tensor("wq", [L, D, D], dtb, kind="ExternalInput")
    wk_i = nc.dram_tensor("wk", [L, D, D], dtb, kind="ExternalInput")
    wv_i = nc.dram_tensor("wv", [L, D, D], dtb, kind="ExternalInput")
    wo_i = nc.dram_tensor("wo", [L, D, D], dtb, kind="ExternalInput")
    w1_i = nc.dram_tensor("w1", [L, D, F], dtb, kind="ExternalInput")
    w2_i = nc.dram_tensor("w2", [L, F, D], dtb, kind="ExternalInput")
    wout_i = nc.dram_tensor("wout", [D, VPAD], dtb, kind="ExternalInput")
    # per-feature params in T layout ([128, n_chunks] per layer)
    g1_i = nc.dram_tensor("g1", [L, 128, 8], dtf, kind="ExternalInput")
    be1_i = nc.dram_tensor("be1", [L, 128, 8], dtf, kind="ExternalInput")
    g2_i = nc.dram_tensor("g2", [L, 128, 8], dtf, kind="ExternalInput")
    be2_i = nc.dram_tensor("be2", [L, 128, 8], dtf, kind="ExternalInput")
    bo_i = nc.dram_tensor("bo", [L, 128, 8], dtf, kind="ExternalInput")
    b1_i = nc.dram_tensor("b1", [L, 128, 32], dtf, kind="ExternalInput")
    b2_i = nc.dram_tensor("b2", [L, 128, 8], dtf, kind="ExternalInput")
    gf_i = nc.dram_tensor("gf", [128, 8], dtf, kind="ExternalInput")
    bf_i = nc.dram_tensor("bf", [128, 8], dtf, kind="ExternalInput")
    bout_i = nc.dram_tensor("bout", [128, 256], dtf, kind="ExternalInput")
    causal_i = nc.dram_tensor("causal", [128, 128], dtf, kind="ExternalInput")
    padb_i = nc.dram_tensor("padb", [2, 8, 128], dtf, kind="ExternalInput")
    out_o = nc.dram_tensor("logits_t", [VPAD, TPC], dtb, kind="ExternalOutput")

    with tile.TileContext(nc) as tc:
        _body(
            nc, tc, mybir, dtr, dtf, dtb, RG, make_identity,
            x0t_i, wq_i, wk_i, wv_i, wo_i, w1_i, w2_i, wout_i,
            g1_i, be1_i, g2_i, be2_i, bo_i, b1_i, b2_i, gf_i, bf_i, bout_i,
            causal_i, padb_i, out_o,
        )
    nc.compile()
    return nc


def _body(nc, tc, mybir, dtr, dtf, dtb, RG, make_identity,
          x0t_i, wq_i, wk_i, wv_i, wo_i, w1_i, w2_i, wout_i,
          g1_i, be1_i, g2_i, be2_i, bo_i, b1_i, b2_i, gf_i, bf_i, bout_i,
          causal_i, padb_i, out_o):
    import contextlib
    AF = mybir.ActivationFunctionType
    OP = mybir.AluOpType
    ctx = contextlib.ExitStack()
    with ctx:
        const = ctx.enter_context(tc.tile_pool(name="const", bufs=1))
        lnp = ctx.enter_context(tc.tile_pool(name="lnp", bufs=1))
        resid = ctx.enter_context(tc.tile_pool(name="resid", bufs=1))
        act = ctx.enter_context(tc.tile_pool(name="act", bufs=1))
        act2 = ctx.enter_context(tc.tile_pool(name="act2", bufs=2))
        expp = ctx.enter_context(tc.tile_pool(name="expp", bufs=2))
        wqkv = ctx.enter_context(tc.tile_pool(name="wqkv", bufs=1))
        wff = ctx.enter_context(tc.tile_pool(name="wff", bufs=2))
        stat = ctx.enter_context(tc.tile_pool(name="stat", bufs=1))
        ps = ctx.enter_context(tc.tile_pool(name="ps", bufs=4, space="PSUM"))
        dram = ctx.enter_context(tc.tile_pool(name="dram", bufs=2, space="DRAM"))

        # ---- constants (staged through one f32 scratch tag) ----
        stage = const.tile([128, 128], dtf, tag="stage")
        nc.vector.memset(stage[:, 0:1], 1.0)
        ones_col = const.tile([128, 1], dtr, tag="ones_col")
        nc.scalar.copy(out=ones_col[:], in_=stage[:, 0:1])
        onesb_col = const.tile([128, 1], dtb, tag="onesb_col")
        nc.scalar.copy(out=onesb_col[:], in_=stage[:, 0:1])
        nc.vector.memset(stage[0:1, :], 1.0)
        ones_row = const.tile([1, 128], dtr, tag="ones_row")
        nc.scalar.copy(out=ones_row[:], in_=stage[0:1, :])
        eps_t = const.tile([1, 1], dtf, tag="eps_t")
        nc.vector.memset(eps_t[:], 1e-5)

        causal = const.tile([128, 128], dtf, tag="causal")
        nc.sync.dma_start(out=causal[:], in_=causal_i[:, :])
        padb = const.tile([128, 16], dtf, tag="padb")
        nc.sync.dma_start(out=padb[:], in_=padb_i.ap().rearrange("b k p -> p (b k)"))

        # ---- persistent residual: feature-major, chunk i = cols [256i, 256i+256) ----
        xT = resid.tile([128, 2048], dtr, tag="xT")
        nc.sync.dma_start(
            out=xT[:].rearrange("p (i t) -> p i t", i=8),
            in_=x0t_i.ap().rearrange("(i p) t -> p i t", p=128),
        )

        def cs(ap, i, w=256):
            return ap[:, i * w:(i + 1) * w]

        def layer_norm(src_tile, g_t, b_t, out_dt):
            """src [128, 2048] f32r -> normalized [128, 2048] out_dt."""
            p_s = ps.tile([128, 1024], dtf, tag="bank")
            for i in range(8):
                sq = act2.tile([128, 256], dtr, tag="ln_sq")
                nc.vector.tensor_mul(out=sq[:], in0=cs(src_tile[:], i),
                                     in1=cs(src_tile[:], i))
                nc.tensor.matmul(p_s[0:1, 0:256], ones_col[:],
                                 cs(src_tile[:], i),
                                 start=(i == 0), stop=(i == 7))
                # start=True clears has_written for the whole bank, so the
                # sq group must NOT restart it: first write lands on cleared
                # bits (overwrite), later writes accumulate.
                nc.tensor.matmul(p_s[0:1, 256:512], ones_col[:], sq[:],
                                 start=False, stop=(i == 7))
            mean_r = stat.tile([1, 256], dtr, tag="mean_r")
            nc.scalar.activation(out=mean_r[:], in_=p_s[0:1, 0:256],
                                 func=AF.Copy, scale=1.0 / 1024.0)
            sc = stat.tile([1, 1024], dtf, tag="sc")  # ex2|msq|var|std
            nc.scalar.activation(out=sc[0:1, 0:256], in_=p_s[0:1, 256:512],
                                 func=AF.Copy, scale=1.0 / 1024.0)
            nc.vector.tensor_mul(out=sc[0:1, 256:512], in0=mean_r[:],
                                 in1=mean_r[:])
            nc.vector.tensor_sub(out=sc[0:1, 512:768], in0=sc[0:1, 0:256],
                                 in1=sc[0:1, 256:512])
            nc.scalar.activation(out=sc[0:1, 768:1024], in_=sc[0:1, 512:768],
                                 func=AF.Sqrt, bias=eps_t[:])
            rstd_r = stat.tile([1, 256], dtr, tag="rstd_r")
            with nc.allow_low_precision(reason="f32r rounding for matmul"):
                nc.vector.reciprocal(out=rstd_r[:], in_=sc[0:1, 768:1024])
            p_b = ps.tile([128, 1024], dtf, tag="bank")
            nc.tensor.matmul(p_b[:, 0:256], ones_row[:], mean_r[:],
                             start=True, stop=True)
            nc.tensor.matmul(p_b[:, 256:512], ones_row[:], rstd_r[:],
                             start=True, stop=True)
            out_t = act.tile([128, 2048], out_dt, tag="ln_out")
            for i in range(8):
                tmp = act2.tile([128, 256], dtr, tag="ln_tmp")
                nc.vector.tensor_sub(out=tmp[:], in0=cs(src_tile[:], i),
                                     in1=p_b[:, 0:256])
                nc.vector.tensor_mul(out=tmp[:], in0=tmp[:],
                                     in1=p_b[:, 256:512])
                nc.vector.tensor_scalar(
                    out=cs(out_t[:], i), in0=tmp[:],
                    scalar1=g_t[:, i:i + 1], scalar2=b_t[:, i:i + 1],
                    op0=OP.mult, op1=OP.add)
            return out_t

        def ln_param(dram_t, tag, idx=None):
            shp = [128, dram_t.shape[-1]]
            t = lnp.tile(shp, dtf, tag=tag)
            nc.sync.dma_start(out=t[:], in_=dram_t[idx] if idx is not None
                              else dram_t[:, :])
            return t

        def probe_dump(slot, tile_ap):
            # tile_ap: [128, 2048] f32-ish, bitcast as bf16 [128, 4096],
            # stored as 16 row-blocks of [128, 256] in out_o.
            bc = tile_ap.bitcast(dtb)
            for i in range(16):
                nc.sync.dma_start(
                    out=out_o[128 * (16 * slot + i):128 * (16 * slot + i + 1), :],
                    in_=bc[:, 256 * i:256 * (i + 1)])

        for l in range(1 if PROBE else L):
            # ---- LN1 -> h (bf16) ----
            hT = layer_norm(xT, ln_param(g1_i, "g1t", l),
                            ln_param(be1_i, "be1t", l), dtb)

            # ---- local QKV on own 256 tokens (full weights, bf16) ----
            wqt = wqkv.tile([128, 8, 1024], dtb, tag="wqt")
            nc.sync.dma_start(out=wqt[:],
                              in_=wq_i[l].rearrange("(c p) m -> p c m", p=128))
            wkt = wqkv.tile([128, 8, 1024], dtb, tag="wkt")
            nc.sync.dma_start(out=wkt[:],
                              in_=wk_i[l].rearrange("(c p) m -> p c m", p=128))
            wvt = wqkv.tile([128, 8, 1024], dtb, tag="wvt")
            nc.sync.dma_start(out=wvt[:],
                              in_=wv_i[l].rearrange("(c p) m -> p c m", p=128))

            q_loc = act.tile([128, 8, 256], dtb, tag="q_loc")
            k_loc = act.tile([128, 8, 256], dtb, tag="k_loc")
            v_loc = act.tile([128, 2, 1024], dtb, tag="v_loc")
            for name, wt, dst in (("q", wqt, q_loc), ("k", wkt, k_loc)):
                for ht in range(2):
                    p_q = ps.tile([128, 1024], dtf, tag="bank")
                    for c in range(8):
                        for m in range(4):
                            nc.tensor.matmul(
                                p_q[:, 256 * m:256 * (m + 1)],
                                wt[:, c, 128 * (4 * ht + m):128 * (4 * ht + m + 1)],
                                cs(hT[:], c),
                                start=(c == 0 and m % 2 == 0), stop=(c == 7))
                    nc.vector.tensor_copy(
                        out=dst[:, 4 * ht:4 * (ht + 1), :].rearrange(
                            "p m t -> p (m t)"),
                        in_=p_q[:])
            # V token-major: pv[tok, vdim] = sum_feat h[feat, tok] wv[feat, vdim]
            for u in range(2):
                p_v = ps.tile([128, 1024], dtf, tag="bank")
                for c in range(8):
                    for s2 in range(2):
                        nc.tensor.matmul(
                            p_v[:, 512 * s2:512 * (s2 + 1)],
                            hT[:, 256 * c + 128 * u:256 * c + 128 * (u + 1)],
                            wvt[:, c, 512 * s2:512 * (s2 + 1)],
                            start=(c == 0), stop=(c == 7))
                nc.scalar.copy(
                    out=v_loc[:, u, :], in_=p_v[:])

            # ---- qkv AllToAll: block j (384 rows) = q|k (feature-major,
            #      dims 128j..) + v (token-major, cols = half0|half1 dims) ----
            a2a_in = dram.tile([NCORES * 384, TPC], dtb, tag="a2a_qkv_in")
            for j in range(NCORES):
                r0 = 384 * j
                nc.sync.dma_start(out=a2a_in[r0:r0 + 128, :],
                                  in_=q_loc[:, j, :])
                nc.sync.dma_start(out=a2a_in[r0 + 128:r0 + 256, :],
                                  in_=k_loc[:, j, :])
                nc.sync.dma_start(
                    out=a2a_in[r0 + 256:r0 + 384, :].rearrange(
                        "p (u d) -> p u d", u=2),
                    in_=v_loc[:, :, 128 * j:128 * (j + 1)])
            a2a_out = dram.tile([NCORES * 384, TPC], dtb, tag="a2a_qkv_out")
            nc.gpsimd.collective_compute(
                "AllToAll", mybir.AluOpType.bypass, replica_groups=RG,
                ins=[a2a_in.opt()], outs=[a2a_out.opt()])

            # receive: my 2 heads (128 dims), all 2048 tokens
            qT = act.tile([128, 2048], dtb, tag="qT")
            kT = act.tile([128, 2048], dtb, tag="kT")
            vN = act.tile([128, 16, 130], dtb, tag="vN")
            for tb in range(16):
                nc.vector.tensor_copy(out=vN[:, tb, 64:65], in_=onesb_col[:])
                nc.vector.tensor_copy(out=vN[:, tb, 129:130], in_=onesb_col[:])
            for j in range(NCORES):
                r0 = 384 * j
                nc.sync.dma_start(out=qT[:, 256 * j:256 * (j + 1)],
                                  in_=a2a_out[r0:r0 + 128, :])
                nc.sync.dma_start(out=kT[:, 256 * j:256 * (j + 1)],
                                  in_=a2a_out[r0 + 128:r0 + 256, :])
                vv = a2a_out[r0 + 256:r0 + 384, :].rearrange(
                    "p (u d) -> p u d", u=2)
                nc.sync.dma_start(out=vN[:, 2 * j:2 * (j + 1), 0:64],
                                  in_=vv[:, :, 0:64])
                nc.sync.dma_start(out=vN[:, 2 * j:2 * (j + 1), 65:129],
                                  in_=vv[:, :, 64:128])

            if PROBE:
                probe_dump(0, hT[:].bitcast(dtf))
                probe_dump(1, qT[:].bitcast(dtf))

            # ---- attention per (head, batch); scores transposed [k, q] ----
            ctx_sb = act.tile([128, 2048], dtb, tag="ctx_sb")
            for hh in range(2):
                for b in range(B):
                    qs = qT[64 * hh:64 * (hh + 1), 1024 * b:1024 * (b + 1)]
                    ks = kT[64 * hh:64 * (hh + 1), 1024 * b:1024 * (b + 1)]
                    p_u = ps.tile([128, 1024], dtf, tag="bank")
                    for kb in range(8):
                        live = 1024 - 128 * kb
                        p_sc = ps.tile([128, 1024], dtf, tag="bank")
                        off = 0
                        while off < live:
                            w = min(512, live - off)
                            nc.tensor.matmul(
                                p_sc[:, off:off + w],
                                ks[:, 128 * kb:128 * (kb + 1)],
                                qs[:, 128 * kb + off:128 * kb + off + w],
                                start=True, stop=True)
                            off += w
                        nc.vector.tensor_add(out=p_sc[:, 0:128],
                                             in0=p_sc[:, 0:128], in1=causal[:])
                        nc.vector.tensor_scalar(
                            out=p_sc[:, 0:live], in0=p_sc[:, 0:live],
                            scalar1=padb[:, 8 * b + kb:8 * b + kb + 1],
                            scalar2=None, op0=OP.add)
                        es = expp.tile([128, 1024], dtb, tag="expS")
                        nc.scalar.activation(out=es[:, 0:live],
                                             in_=p_sc[:, 0:live], func=AF.Exp)
                        # U = [1 | V].T @ expS accumulated over k blocks
                        vsl = vN[:, 8 * b + kb, 65 * hh:65 * (hh + 1)]
                        off = 0
                        while off < live:
                            w = min(512, live - off)
                            nc.tensor.matmul(
                                p_u[0:65, 128 * kb + off:128 * kb + off + w],
                                vsl, es[:, off:off + w],
                                start=(kb == 0), stop=(kb == 7))
                            off += w
                    # rows: 0:64 = unnormalized ctx, 64 = sum(exp)
                    rc = stat.tile([1, 1024], dtb, tag="rc")
                    with nc.allow_low_precision(reason="softmax norm bf16"):
                        nc.vector.reciprocal(out=rc[:], in_=p_u[64:65, :])
                    rbb = stat.tile([64, 1024], dtb, tag="rbb")
                    nc.gpsimd.partition_broadcast(rbb[:], rc[:])
                    nc.vector.tensor_mul(
                        out=ctx_sb[64 * hh:64 * (hh + 1),
                                   1024 * b:1024 * (b + 1)],
                        in0=p_u[0:64, :], in1=rbb[:])

            # ---- ctx AllToAll: shard j = my heads x rank-j tokens ----
            a2a2_in = dram.tile([NCORES * 128, TPC], dtb, tag="a2a_ctx_in")
            for j in range(NCORES):
                nc.sync.dma_start(out=a2a2_in[128 * j:128 * (j + 1), :],
                                  in_=cs(ctx_sb[:], j))
            a2a2_out = dram.tile([NCORES * 128, TPC], dtb, tag="a2a_ctx_out")
            nc.gpsimd.collective_compute(
                "AllToAll", mybir.AluOpType.bypass, replica_groups=RG,
                ins=[a2a2_in.opt()], outs=[a2a2_out.opt()])
            ctxf = act.tile([128, 8, 256], dtb, tag="ctxf")
            nc.sync.dma_start(
                out=ctxf[:],
                in_=a2a2_out[:, :].rearrange("(c p) t -> p c t", p=128))

            # ---- Wo + bias + residual (own tokens) ----
            bot = ln_param(bo_i, "bot", l)
            wot = wff.tile([128, 8, 1024], dtb, tag="w1gt")
            nc.sync.dma_start(
                out=wot[:], in_=wo_i[l].rearrange("(c p) n -> p c n", p=128))
            for m in range(8):
                p_y = ps.tile([128, 1024], dtf, tag="bank")
                for c in range(8):
                    nc.tensor.matmul(p_y[:, 0:256],
                                     wot[:, c, 128 * m:128 * (m + 1)],
                                     ctxf[:, c, :],
                                     start=(c == 0), stop=(c == 7))
                tmp = act2.tile([128, 256], dtr, tag="res_tmp")
                nc.vector.tensor_scalar(
                    out=tmp[:], in0=p_y[:, 0:256], scalar1=bot[:, m:m + 1],
                    scalar2=None, op0=OP.add)
                nc.vector.tensor_add(out=cs(xT[:], m), in0=cs(xT[:], m),
                                     in1=tmp[:])

            if PROBE:
                probe_dump(2, ctx_sb[:].bitcast(dtf))
                probe_dump(3, xT[:].bitcast(dtf))

            # ---- LN2 + FFN ----
            h2T = layer_norm(xT, ln_param(g2_i, "g2t", l),
                             ln_param(be2_i, "be2t", l), dtb)
            b1t = ln_param(b1_i, "b1t", l)
            # y2 accumulates in PSUM across all 4 F-groups
            p_y2a = ps.tile([128, 1024], dtf, tag="bank")
            p_y2b = ps.tile([128, 1024], dtf, tag="bank")
            for g in range(4):
                w1gt = wff.tile([128, 8, 1024], dtb, tag="w1gt")
                nc.sync.dma_start(
                    out=w1gt[:],
                    in_=w1_i[l][:, 1024 * g:1024 * (g + 1)].rearrange(
                        "(c p) f -> p c f", p=128))
                w2gt = wff.tile([128, 8, 1024], dtb, tag="w2gt")
                nc.sync.dma_start(
                    out=w2gt[:],
                    in_=w2_i[l][1024 * g:1024 * (g + 1), :].rearrange(
                        "(f p) m -> p f m", p=128))
                p_u1 = ps.tile([128, 1024], dtf, tag="bank")
                p_u2 = ps.tile([128, 1024], dtf, tag="bank")
                for c in range(8):
                    for fl in range(8):
                        pu = p_u1 if fl < 4 else p_u2
                        nc.tensor.matmul(
                            pu[:, 256 * (fl % 4):256 * (fl % 4 + 1)],
                            w1gt[:, c, 128 * fl:128 * (fl + 1)], cs(h2T[:], c),
                            start=(c == 0 and fl % 2 == 0), stop=(c == 7))
                guT = act2.tile([128, 2048], dtb, tag="guT")
                for fl in range(8):
                    pu = p_u1 if fl < 4 else p_u2
                    fc = 8 * g + fl
                    nc.scalar.activation(
                        out=cs(guT[:], fl),
                        in_=pu[:, 256 * (fl % 4):256 * (fl % 4 + 1)],
                        func=AF.Gelu, bias=b1t[:, fc:fc + 1])
                for fl in range(8):
                    for mq in range(8):
                        py = p_y2a if mq < 4 else p_y2b
                        nc.tensor.matmul(
                            py[:, 256 * (mq % 4):256 * (mq % 4 + 1)],
                            w2gt[:, fl, 128 * mq:128 * (mq + 1)], cs(guT[:], fl),
                            start=(g == 0 and fl == 0 and mq % 2 == 0),
                            stop=(g == 3 and fl == 7))
            b2t = ln_param(b2_i, "b2t", l)
            for m in range(8):
                py = p_y2a if m < 4 else p_y2b
                tmp2 = act2.tile([128, 256], dtr, tag="res_tmp")
                nc.vector.tensor_scalar(
                    out=tmp2[:], in0=py[:, 256 * (m % 4):256 * (m % 4 + 1)],
                    scalar1=b2t[:, m:m + 1], scalar2=None, op0=OP.add)
                nc.vector.tensor_add(out=cs(xT[:], m), in0=cs(xT[:], m),
                                     in1=tmp2[:])

        if PROBE:
            probe_dump(4, xT[:].bitcast(dtf))
            return

        # ---- final LN + token-sharded LM head (no collective) ----
        xfT = layer_norm(xT, ln_param(gf_i, "gft"), ln_param(bf_i, "bft"), dtb)
        boutt = ln_param(bout_i, "boutt")
        for vg in range(VPAD // 1024):
            wvh = wqkv.tile([128, 8, 1024], dtb,
                            tag=("wqt", "wkt", "wvt")[vg % 3])
            nc.sync.dma_start(
                out=wvh[:],
                in_=wout_i.ap()[:, 1024 * vg:1024 * (vg + 1)].rearrange(
                    "(c p) m -> p c m", p=128))
            osb = act2.tile([128, 2048], dtb, tag="osb")
            for q8 in range(8):
                vm = 8 * vg + q8
                p_o = ps.tile([128, 1024], dtf, tag="bank")
                for c in range(8):
                    nc.tensor.matmul(
                        p_o[:, 0:256], wvh[:, c, 128 * q8:128 * (q8 + 1)],
                        cs(xfT[:], c),
                        start=(c == 0), stop=(c == 7))
                nc.vector.tensor_scalar(
                    out=osb[:, 256 * q8:256 * (q8 + 1)], in0=p_o[:, 0:256],
                    scalar1=boutt[:, vm:vm + 1],
                    scalar2=None, op0=OP.add)
            nc.sync.dma_start(
                out=out_o[1024 * vg:1024 * (vg + 1), :].rearrange(
                    "(q p) t -> p q t", p=128),
                in_=osb[:].rearrange("p (q t) -> p q t", q=8))


def _host_inputs(inputs):
    bf16 = ml_dtypes.bfloat16
    tokens = np.asarray(inputs["tokens"])
    emb = np.asarray(inputs["emb"], dtype=np.float32)
    pe = np.asarray(inputs["pe"], dtype=np.float32)
    Wq = np.asarray(inputs["Wq"], dtype=np.float32)
    Wk = np.asarray(inputs["Wk"], dtype=np.float32)
    Wv = np.asarray(inputs["Wv"], dtype=np.float32)
    Wo = np.asarray(inputs["Wo"], dtype=np.float32)
    bo = np.asarray(inputs["bo"], dtype=np.float32)
    g1 = np.asarray(inputs["g1"], dtype=np.float32)
    be1 = np.asarray(inputs["be1"], dtype=np.float32)
    g2 = np.asarray(inputs["g2"], dtype=np.float32)
    be2 = np.asarray(inputs["be2"], dtype=np.float32)
    W1 = np.asarray(inputs["W1"], dtype=np.float32)
    b1 = np.asarray(inputs["b1"], dtype=np.float32)
    W2 = np.asarray(inputs["W2"], dtype=np.float32)
    b2 = np.asarray(inputs["b2"], dtype=np.float32)
    gf = np.asarray(inputs["gf"], dtype=np.float32)
    bf = np.asarray(inputs["bf"], dtype=np.float32)
    Wout = np.asarray(inputs["Wout"], dtype=np.float32)
    bout = np.asarray(inputs["bout"], dtype=np.float32)

    x0 = emb[tokens] * math.sqrt(float(D)) + pe[:S][None]   # (B, S, D)
    xflat = np.ascontiguousarray(x0.reshape(T, D))

    padb = np.where(tokens == 0, np.float32(NEG), np.float32(0.0))
    padb = np.ascontiguousarray(padb.reshape(2, 8, 128).astype(np.float32))
    r = np.arange(128)
    causal = np.where(r[:, None] > r[None, :], np.float32(NEG),
                      np.float32(0.0)).astype(np.float32)

    def tchunks(a, n):   # [L, D'] -> [L, 128, n] feature-major chunks
        return np.ascontiguousarray(
            a.reshape(L, n, 128).transpose(0, 2, 1).astype(np.float32))

    g1t, be1t = tchunks(g1, 8), tchunks(be1, 8)
    g2t, be2t = tchunks(g2, 8), tchunks(be2, 8)
    bot, b2t = tchunks(bo, 8), tchunks(b2, 8)
    b1t = tchunks(b1, 32)
    gft = np.ascontiguousarray(gf.reshape(8, 128).T.astype(np.float32))
    bft = np.ascontiguousarray(bf.reshape(8, 128).T.astype(np.float32))

    wout_p = np.zeros((D, VPAD), dtype=np.float32)
    wout_p[:, :V] = Wout
    bout_p = np.zeros((VPAD,), dtype=np.float32)
    bout_p[:V] = bout

    wq_s = (Wq / math.sqrt(float(DK))).astype(np.float32)

    shared = dict(
        wq=np.ascontiguousarray(wq_s.astype(bf16)),
        wk=np.ascontiguousarray(Wk.astype(bf16)),
        wv=np.ascontiguousarray(Wv.astype(bf16)),
        wo=np.ascontiguousarray(Wo.astype(bf16)),
        w1=np.ascontiguousarray(W1.astype(bf16)),
        w2=np.ascontiguousarray(W2.astype(bf16)),
        wout=np.ascontiguousarray(wout_p.astype(bf16)),
        bout=np.ascontiguousarray(bout_p.reshape(256, 128).T.astype(np.float32)),
        g1=g1t, be1=be1t, g2=g2t, be2=be2t, bo=bot, b1=b1t, b2=b2t,
        gf=gft, bf=bft, causal=causal, padb=padb,
    )
    in_maps = []
    for c in range(NCORES):
        m = dict(shared)
        m["x0t"] = np.ascontiguousarray(
            xflat[TPC * c:TPC * (c + 1)].T)
        in_maps.append(m)
    return in_maps


def kernel(**inputs):
    from concourse import bass_utils

    if "nc" not in _CACHE:
        _CACHE["nc"] = _build()
    nc = _CACHE["nc"]
    in_maps = _host_inputs(inputs)
    res = bass_utils.run_bass_kernel_spmd(
        nc, in_maps, core_ids=list(range(NCORES)))
    full = np.empty((T, V), dtype=np.float32)
    for c in range(NCORES):
        lt = res.results[c]["logits_t"]          # [VPAD, TPC] bf16
        full[TPC * c:TPC * (c + 1), :] = lt[:V, :].T.astype(np.float32)
    return full.reshape(B, S, V)


if __name__ == "__main__":
    import sys
    sys.path.insert(0, "/root/problem")
    import reference
    inp = reference.setup_inputs()
    out = kernel(**{k: np.asarray(v) for k, v in inp.items()})
    print("kernel output", out.shape, out.dtype)


# revision 15
# speedup vs baseline: 1.8084x; 1.6695x over previous
"""Decoder-only transformer forward on 8 trn2 NeuronCores.

Sharding (SPMD, two small AllToAlls per layer, no AllGather):
  - residual stream token-sharded: core c owns flat tokens [256c, 256c+256)
  - QKV computed locally on own tokens with FULL Wq/Wk/Wv (bf16), then one
    AllToAll redistributes q,k,v head-sharded: core c gets heads (2c, 2c+1)
    for all 2048 tokens (1.5 MB bf16 per rank)
  - attention head-sharded; ctx AllToAll'd back token-sharded (0.5 MB bf16)
  - Wo / FFN token-sharded (full bf16 weights streamed per core)
  - LM head token-sharded: full Wout (bf16) streamed per core, logits out
    bf16 — no final collective.

Layouts: activations feature-major ("T": [d partitions, tokens free]); V is
produced token-major directly by swapping matmul operands (no PE
transposes); scores computed transposed ([k, q]) so the pad-key bias is a
per-partition scalar and V-hat's appended ones-row yields the softmax
normalizer from the same accumulation.

Matmul weights/activations bf16 (full PE rate, half DMA + A2A wire);
PSUM accumulation fp32; residual stream fp32r; LN stats via PE
ones-matmul partition reductions.
"""

import math
import os

import numpy as np
import ml_dtypes

PROBE = bool(os.environ.get("BASS_PROBE"))

B, S, D, H, L, F, V = 2, 1024, 1024, 16, 6, 4096, 32000
NCORES = 8
T = B * S                 # 2048 flat tokens
TPC = T // NCORES         # 256 tokens per core
VPAD = 32768
DK = D // H               # 64
NEG = -1e9
EPS = 1e-5

_CACHE = {}


def _build():
    import concourse.mybir as mybir
    import concourse.tile as tile
    from concourse import bacc
    from concourse.masks import make_identity

    dtr = mybir.dt.float32r
    dtf = mybir.dt.float32
    dtb = mybir.dt.bfloat16

    nc = bacc.Bacc(
        "TRN2",
        target_bir_lowering=False,
        debug=False,
        enable_asserts=False,
        num_devices=NCORES,
    )
    RG = [list(range(NCORES))]

    # ---- I/O ----
    x0t_i = nc.dram_tensor("x0t", [D, TPC], dtr, kind="ExternalInput")
    wq_i = nc.dram_tensor("wq", [L, D, D], dtb, kind="ExternalInput")
    wk_i = nc.dram_tensor("wk", [L, D, D], dtb, kind="ExternalInput")
    wv_i = nc.dram_tensor("wv", [L, D, D], dtb, kind="ExternalInput")
    wo_i = nc.dram_tensor("wo", [L, D, D], dtb, kind="ExternalInput")
    w1_i = nc.dram_tensor("w1", [L, D, F], dtb, kind="ExternalInput")
    w2_i = nc.dram_tensor("w2", [L, F, D], dtb, kind="ExternalInput")
    wout_i = nc.dram_tensor("wout", [D, VPAD], dtb, kind="ExternalInput")
    # per-feature params in T layout ([128, n_chunks] per layer)
    g1_i = nc.dram_tensor("g1", [L, 128, 8], dtf, kind="ExternalInput")
    be1_i = nc.dram_tensor("be1", [L, 128, 8], dtf, kind="ExternalInput")
    g2_i = nc.dram_tensor("g2", [L, 128, 8], dtf, kind="ExternalInput")
    be2_i = nc.dram_tensor("be2", [L, 128, 8], dtf, kind="ExternalInput")
    bo_i = nc.dram_tensor("bo", [L, 128, 8], dtf, kind="ExternalInput")
    b1_i = nc.dram_tensor("b1", [L, 128, 32], dtf, kind="ExternalInput")
    b2_i = nc.dram_tensor("b2", [L, 128, 8], dtf, kind="ExternalInput")
    gf_i = nc.dram_tensor("gf", [128, 8], dtf, kind="ExternalInput")
    bf_i = nc.dram_tensor("bf", [128, 8], dtf, kind="ExternalInput")
    bout_i = nc.dram_tensor("bout", [128, 256], dtf, kind="ExternalInput")
    causal_i = nc.dram_tensor("causal", [128, 128], dtf, kind="ExternalInput")
    padb_i = nc.dram_tensor("padb", [2, 8, 128], dtf, kind="ExternalInput")
    # logits packed [128, 65536]: vocab v = 1024*vg + 128*q8 + p lives at
    # row p, col 2048*vg + 256*q8 + t  (4 KB DMA lines instead of 512 B)
    out_o = nc.dram_tensor("logits_t", [128, VPAD * TPC // 128], dtb,
                           kind="ExternalOutput")

    with tile.TileContext(nc) as tc:
        _body(
            nc, tc, mybir, dtr, dtf, dtb, RG, make_identity,
            x0t_i, wq_i, wk_i, wv_i, wo_i, w1_i, w2_i, wout_i,
            g1_i, be1_i, g2_i, be2_i, bo_i, b1_i, b2_i, gf_i, bf_i, bout_i,
            causal_i, padb_i, out_o,
        )
    nc.compile()
    return nc


def _body(nc, tc, mybir, dtr, dtf, dtb, RG, make_identity,
          x0t_i, wq_i, wk_i, wv_i, wo_i, w1_i, w2_i, wout_i,
          g1_i, be1_i, g2_i, be2_i, bo_i, b1_i, b2_i, gf_i, bf_i, bout_i,
          causal_i, padb_i, out_o):
    import contextlib
    AF = mybir.ActivationFunctionType
    OP = mybir.AluOpType
    ctx = contextlib.ExitStack()
    with ctx:
        const = ctx.enter_context(tc.tile_pool(name="const", bufs=1))
        lnp = ctx.enter_context(tc.tile_pool(name="lnp", bufs=1))
        resid = ctx.enter_context(tc.tile_pool(name="resid", bufs=1))
        act = ctx.enter_context(tc.tile_pool(name="act", bufs=1))
        act2 = ctx.enter_context(tc.tile_pool(name="act2", bufs=2))
        expp = ctx.enter_context(tc.tile_pool(name="expp", bufs=2))
        wqkv = ctx.enter_context(tc.tile_pool(name="wqkv", bufs=1))
        wff = ctx.enter_context(tc.tile_pool(name="wff", bufs=2))
        stat = ctx.enter_context(tc.tile_pool(name="stat", bufs=1))
        ps = ctx.enter_context(tc.tile_pool(name="ps", bufs=4, space="PSUM"))
        dram = ctx.enter_context(tc.tile_pool(name="dram", bufs=2, space="DRAM"))

        # ---- constants (staged through one f32 scratch tag) ----
        stage = const.tile([128, 128], dtf, tag="stage")
        nc.vector.memset(stage[:, 0:1], 1.0)
        ones_col = const.tile([128, 1], dtr, tag="ones_col")
        nc.scalar.copy(out=ones_col[:], in_=stage[:, 0:1])
        onesb_col = const.tile([128, 1], dtb, tag="onesb_col")
        nc.scalar.copy(out=onesb_col[:], in_=stage[:, 0:1])
        nc.vector.memset(stage[0:1, :], 1.0)
        ones_row = const.tile([1, 128], dtr, tag="ones_row")
        nc.scalar.copy(out=ones_row[:], in_=stage[0:1, :])
        eps_t = const.tile([1, 1], dtf, tag="eps_t")
        nc.vector.memset(eps_t[:], 1e-5)

        causal = const.tile([128, 128], dtf, tag="causal")
        nc.sync.dma_start(out=causal[:], in_=causal_i[:, :])
        padb = const.tile([128, 16], dtf, tag="padb")
        nc.sync.dma_start(out=padb[:], in_=padb_i.ap().rearrange("b k p -> p (b k)"))

        # ---- persistent residual: feature-major, chunk i = cols [256i, 256i+256) ----
        xT = resid.tile([128, 2048], dtr, tag="xT")
        nc.sync.dma_start(
            out=xT[:].rearrange("p (i t) -> p i t", i=8),
            in_=x0t_i.ap().rearrange("(i p) t -> p i t", p=128),
        )

        def cs(ap, i, w=256):
            return ap[:, i * w:(i + 1) * w]

        def layer_norm(src_tile, g_t, b_t, out_dt):
            """src [128, 2048] f32r -> normalized [128, 2048] out_dt."""
            p_s = ps.tile([128, 1024], dtf, tag="bank")
            for i in range(8):
                sq = act2.tile([128, 256], dtr, tag="ln_sq")
                nc.vector.tensor_mul(out=sq[:], in0=cs(src_tile[:], i),
                                     in1=cs(src_tile[:], i))
                nc.tensor.matmul(p_s[0:1, 0:256], ones_col[:],
                                 cs(src_tile[:], i),
                                 start=(i == 0), stop=(i == 7))
                # start=True clears has_written for the whole bank, so the
                # sq group must NOT restart it: first write lands on cleared
                # bits (overwrite), later writes accumulate.
                nc.tensor.matmul(p_s[0:1, 256:512], ones_col[:], sq[:],
                                 start=False, stop=(i == 7))
            mean_r = stat.tile([1, 256], dtr, tag="mean_r")
            nc.scalar.activation(out=mean_r[:], in_=p_s[0:1, 0:256],
                                 func=AF.Copy, scale=1.0 / 1024.0)
            sc = stat.tile([1, 1024], dtf, tag="sc")  # ex2|msq|var|std
            nc.scalar.activation(out=sc[0:1, 0:256], in_=p_s[0:1, 256:512],
                                 func=AF.Copy, scale=1.0 / 1024.0)
            nc.vector.tensor_mul(out=sc[0:1, 256:512], in0=mean_r[:],
                                 in1=mean_r[:])
            nc.vector.tensor_sub(out=sc[0:1, 512:768], in0=sc[0:1, 0:256],
                                 in1=sc[0:1, 256:512])
            nc.scalar.activation(out=sc[0:1, 768:1024], in_=sc[0:1, 512:768],
                                 func=AF.Sqrt, bias=eps_t[:])
            rstd_r = stat.tile([1, 256], dtr, tag="rstd_r")
            with nc.allow_low_precision(reason="f32r rounding for matmul"):
                nc.vector.reciprocal(out=rstd_r[:], in_=sc[0:1, 768:1024])
            p_b = ps.tile([128, 1024], dtf, tag="bank")
            nc.tensor.matmul(p_b[:, 0:256], ones_row[:], mean_r[:],
                             start=True, stop=True)
            nc.tensor.matmul(p_b[:, 256:512], ones_row[:], rstd_r[:],
                             start=True, stop=True)
            out_t = act.tile([128, 2048], out_dt, tag="ln_out")
            for i in range(8):
                tmp = act2.tile([128, 256], dtr, tag="ln_tmp")
                nc.vector.tensor_sub(out=tmp[:], in0=cs(src_tile[:], i),
                                     in1=p_b[:, 0:256])
                nc.vector.tensor_mul(out=tmp[:], in0=tmp[:],
                                     in1=p_b[:, 256:512])
                nc.vector.tensor_scalar(
                    out=cs(out_t[:], i), in0=tmp[:],
                    scalar1=g_t[:, i:i + 1], scalar2=b_t[:, i:i + 1],
                    op0=OP.mult, op1=OP.add)
            return out_t

        def ln_param(dram_t, tag, idx=None):
            shp = [128, dram_t.shape[-1]]
            t = lnp.tile(shp, dtf, tag=tag)
            nc.sync.dma_start(out=t[:], in_=dram_t[idx] if idx is not None
                              else dram_t[:, :])
            return t

        def probe_dump(slot, tile_ap):
            # tile_ap: [128, 2048] f32-ish, bitcast as bf16 [128, 4096]
            bc = tile_ap.bitcast(dtb)
            nc.sync.dma_start(
                out=out_o[:, 4096 * slot:4096 * (slot + 1)], in_=bc[:])

        for l in range(1 if PROBE else L):
            # ---- LN1 -> h (bf16) ----
            hT = layer_norm(xT, ln_param(g1_i, "g1t", l),
                            ln_param(be1_i, "be1t", l), dtb)

            # ---- local QKV on own 256 tokens (full weights, bf16) ----
            wqt = wqkv.tile([128, 8, 1024], dtb, tag="wqt")
            nc.sync.dma_start(out=wqt[:],
                              in_=wq_i[l].rearrange("(c p) m -> p c m", p=128))
            wkt = wqkv.tile([128, 8, 1024], dtb, tag="wkt")
            nc.sync.dma_start(out=wkt[:],
                              in_=wk_i[l].rearrange("(c p) m -> p c m", p=128))
            wvt = wqkv.tile([128, 8, 1024], dtb, tag="wvt")
            nc.sync.dma_start(out=wvt[:],
                              in_=wv_i[l].rearrange("(c p) m -> p c m", p=128))

            q_loc = act.tile([128, 8, 256], dtb, tag="q_loc")
            k_loc = act.tile([128, 8, 256], dtb, tag="k_loc")
            v_loc = act.tile([128, 2, 1024], dtb, tag="v_loc")
            for name, wt, dst in (("q", wqt, q_loc), ("k", wkt, k_loc)):
                for ht in range(2):
                    p_q = ps.tile([128, 1024], dtf, tag="bank")
                    for c in range(8):
                        for m in range(4):
                            nc.tensor.matmul(
                                p_q[:, 256 * m:256 * (m + 1)],
                                wt[:, c, 128 * (4 * ht + m):128 * (4 * ht + m + 1)],
                                cs(hT[:], c),
                                start=(c == 0 and m % 2 == 0), stop=(c == 7))
                    nc.vector.tensor_copy(
                        out=dst[:, 4 * ht:4 * (ht + 1), :].rearrange(
                            "p m t -> p (m t)"),
                        in_=p_q[:])
            # V token-major: pv[tok, vdim] = sum_feat h[feat, tok] wv[feat, vdim]
            for u in range(2):
                p_v = ps.tile([128, 1024], dtf, tag="bank")
                for c in range(8):
                    for s2 in range(2):
                        nc.tensor.matmul(
                            p_v[:, 512 * s2:512 * (s2 + 1)],
                            hT[:, 256 * c + 128 * u:256 * c + 128 * (u + 1)],
                            wvt[:, c, 512 * s2:512 * (s2 + 1)],
                            start=(c == 0), stop=(c == 7))
                nc.scalar.copy(
                    out=v_loc[:, u, :], in_=p_v[:])

            # ---- qkv AllToAll: block j (384 rows) = q|k (feature-major,
            #      dims 128j..) + v (token-major, cols = half0|half1 dims) ----
            a2a_in = dram.tile([NCORES * 384, TPC], dtb, tag="a2a_qkv_in")
            for j in range(NCORES):
                r0 = 384 * j
                nc.sync.dma_start(out=a2a_in[r0:r0 + 128, :],
                                  in_=q_loc[:, j, :])
                nc.sync.dma_start(out=a2a_in[r0 + 128:r0 + 256, :],
                                  in_=k_loc[:, j, :])
                nc.sync.dma_start(
                    out=a2a_in[r0 + 256:r0 + 384, :].rearrange(
                        "p (u d) -> p u d", u=2),
                    in_=v_loc[:, :, 128 * j:128 * (j + 1)])
            a2a_out = dram.tile([NCORES * 384, TPC], dtb, tag="a2a_qkv_out")
            nc.gpsimd.collective_compute(
                "AllToAll", mybir.AluOpType.bypass, replica_groups=RG,
                ins=[a2a_in.opt()], outs=[a2a_out.opt()])

            # receive: my 2 heads (128 dims), all 2048 tokens
            qT = act.tile([128, 2048], dtb, tag="qT")
            kT = act.tile([128, 2048], dtb, tag="kT")
            vN = act.tile([128, 16, 130], dtb, tag="vN")
            for tb in range(16):
                nc.vector.tensor_copy(out=vN[:, tb, 64:65], in_=onesb_col[:])
                nc.vector.tensor_copy(out=vN[:, tb, 129:130], in_=onesb_col[:])
            for j in range(NCORES):
                r0 = 384 * j
                nc.sync.dma_start(out=qT[:, 256 * j:256 * (j + 1)],
                                  in_=a2a_out[r0:r0 + 128, :])
                nc.sync.dma_start(out=kT[:, 256 * j:256 * (j + 1)],
                                  in_=a2a_out[r0 + 128:r0 + 256, :])
                vv = a2a_out[r0 + 256:r0 + 384, :].rearrange(
                    "p (u d) -> p u d", u=2)
                nc.sync.dma_start(out=vN[:, 2 * j:2 * (j + 1), 0:64],
                                  in_=vv[:, :, 0:64])
                nc.sync.dma_start(out=vN[:, 2 * j:2 * (j + 1), 65:129],
                                  in_=vv[:, :, 64:128])

            if PROBE:
                probe_dump(0, hT[:].bitcast(dtf))
                probe_dump(1, qT[:].bitcast(dtf))

            # ---- attention per (head, batch); scores transposed [k, q] ----
            ctx_sb = act.tile([128, 2048], dtb, tag="ctx_sb")
            for hh in range(2):
                for b in range(B):
                    qs = qT[64 * hh:64 * (hh + 1), 1024 * b:1024 * (b + 1)]
                    ks = kT[64 * hh:64 * (hh + 1), 1024 * b:1024 * (b + 1)]
                    p_u = ps.tile([128, 1024], dtf, tag="bank")
                    for kb in range(8):
                        live = 1024 - 128 * kb
                        p_sc = ps.tile([128, 1024], dtf, tag="bank")
                        off = 0
                        while off < live:
                            w = min(512, live - off)
                            nc.tensor.matmul(
                                p_sc[:, off:off + w],
                                ks[:, 128 * kb:128 * (kb + 1)],
                                qs[:, 128 * kb + off:128 * kb + off + w],
                                start=True, stop=True)
                            off += w
                        nc.vector.tensor_add(out=p_sc[:, 0:128],
                                             in0=p_sc[:, 0:128], in1=causal[:])
                        nc.vector.tensor_scalar(
                            out=p_sc[:, 0:live], in0=p_sc[:, 0:live],
                            scalar1=padb[:, 8 * b + kb:8 * b + kb + 1],
                            scalar2=None, op0=OP.add)
                        es = expp.tile([128, 1024], dtb, tag="expS")
                        nc.scalar.activation(out=es[:, 0:live],
                                             in_=p_sc[:, 0:live], func=AF.Exp)
                        # U = [1 | V].T @ expS accumulated over k blocks
                        vsl = vN[:, 8 * b + kb, 65 * hh:65 * (hh + 1)]
                        off = 0
                        while off < live:
                            w = min(512, live - off)
                            nc.tensor.matmul(
                                p_u[0:65, 128 * kb + off:128 * kb + off + w],
                                vsl, es[:, off:off + w],
                                start=(kb == 0), stop=(kb == 7))
                            off += w
                    # rows: 0:64 = unnormalized ctx, 64 = sum(exp)
                    rc = stat.tile([1, 1024], dtb, tag="rc")
                    with nc.allow_low_precision(reason="softmax norm bf16"):
                        nc.vector.reciprocal(out=rc[:], in_=p_u[64:65, :])
                    rbb = stat.tile([64, 1024], dtb, tag="rbb")
                    nc.gpsimd.partition_broadcast(rbb[:], rc[:])
                    nc.vector.tensor_mul(
                        out=ctx_sb[64 * hh:64 * (hh + 1),
                                   1024 * b:1024 * (b + 1)],
                        in0=p_u[0:64, :], in1=rbb[:])

            # ---- ctx AllToAll: shard j = my heads x rank-j tokens ----
            a2a2_in = dram.tile([NCORES * 128, TPC], dtb, tag="a2a_ctx_in")
            for j in range(NCORES):
                nc.sync.dma_start(out=a2a2_in[128 * j:128 * (j + 1), :],
                                  in_=cs(ctx_sb[:], j))
            a2a2_out = dram.tile([NCORES * 128, TPC], dtb, tag="a2a_ctx_out")
            nc.gpsimd.collective_compute(
                "AllToAll", mybir.AluOpType.bypass, replica_groups=RG,
                ins=[a2a2_in.opt()], outs=[a2a2_out.opt()])
            ctxf = act.tile([128, 8, 256], dtb, tag="ctxf")
            nc.sync.dma_start(
                out=ctxf[:],
                in_=a2a2_out[:, :].rearrange("(c p) t -> p c t", p=128))

            # ---- Wo + bias + residual (own tokens) ----
            bot = ln_param(bo_i, "bot", l)
            wot = wff.tile([128, 8, 1024], dtb, tag="w1gt")
            nc.sync.dma_start(
                out=wot[:], in_=wo_i[l].rearrange("(c p) n -> p c n", p=128))
            for m in range(8):
                p_y = ps.tile([128, 1024], dtf, tag="bank")
                for c in range(8):
                    nc.tensor.matmul(p_y[:, 0:256],
                                     wot[:, c, 128 * m:128 * (m + 1)],
                                     ctxf[:, c, :],
                                     start=(c == 0), stop=(c == 7))
                tmp = act2.tile([128, 256], dtr, tag="res_tmp")
                nc.vector.tensor_scalar(
                    out=tmp[:], in0=p_y[:, 0:256], scalar1=bot[:, m:m + 1],
                    scalar2=None, op0=OP.add)
                nc.vector.tensor_add(out=cs(xT[:], m), in0=cs(xT[:], m),
                                     in1=tmp[:])

            if PROBE:
                probe_dump(2, ctx_sb[:].bitcast(dtf))
                probe_dump(3, xT[:].bitcast(dtf))

            # ---- LN2 + FFN ----
            h2T = layer_norm(xT, ln_param(g2_i, "g2t", l),
                             ln_param(be2_i, "be2t", l), dtb)
            b1t = ln_param(b1_i, "b1t", l)
            # y2 accumulates in PSUM across all 4 F-groups
            p_y2a = ps.tile([128, 1024], dtf, tag="bank")
            p_y2b = ps.tile([128, 1024], dtf, tag="bank")
            for g in range(4):
                w1gt = wff.tile([128, 8, 1024], dtb, tag="w1gt")
                nc.sync.dma_start(
                    out=w1gt[:],
                    in_=w1_i[l][:, 1024 * g:1024 * (g + 1)].rearrange(
                        "(c p) f -> p c f", p=128))
                w2gt = wff.tile([128, 8, 1024], dtb, tag="w2gt")
                nc.sync.dma_start(
                    out=w2gt[:],
                    in_=w2_i[l][1024 * g:1024 * (g + 1), :].rearrange(
                        "(f p) m -> p f m", p=128))
                p_u1 = ps.tile([128, 1024], dtf, tag="bank")
                p_u2 = ps.tile([128, 1024], dtf, tag="bank")
                for c in range(8):
                    for fl in range(8):
                        pu = p_u1 if fl < 4 else p_u2
                        nc.tensor.matmul(
                            pu[:, 256 * (fl % 4):256 * (fl % 4 + 1)],
                            w1gt[:, c, 128 * fl:128 * (fl + 1)], cs(h2T[:], c),
                            start=(c == 0 and fl % 2 == 0), stop=(c == 7))
                guT = act2.tile([128, 2048], dtb, tag="guT")
                for fl in range(8):
                    pu = p_u1 if fl < 4 else p_u2
                    fc = 8 * g + fl
                    nc.scalar.activation(
                        out=cs(guT[:], fl),
                        in_=pu[:, 256 * (fl % 4):256 * (fl % 4 + 1)],
                        func=AF.Gelu, bias=b1t[:, fc:fc + 1])
                for fl in range(8):
                    for mq in range(8):
                        py = p_y2a if mq < 4 else p_y2b
                        nc.tensor.matmul(
                            py[:, 256 * (mq % 4):256 * (mq % 4 + 1)],
                            w2gt[:, fl, 128 * mq:128 * (mq + 1)], cs(guT[:], fl),
                            start=(g == 0 and fl == 0 and mq % 2 == 0),
                            stop=(g == 3 and fl == 7))
            b2t = ln_param(b2_i, "b2t", l)
            for m in range(8):
                py = p_y2a if m < 4 else p_y2b
                tmp2 = act2.tile([128, 256], dtr, tag="res_tmp")
                nc.vector.tensor_scalar(
                    out=tmp2[:], in0=py[:, 256 * (m % 4):256 * (m % 4 + 1)],
                    scalar1=b2t[:, m:m + 1], scalar2=None, op0=OP.add)
                nc.vector.tensor_add(out=cs(xT[:], m), in0=cs(xT[:], m),
                                     in1=tmp2[:])

        if PROBE:
            probe_dump(4, xT[:].bitcast(dtf))
            return

        # ---- final LN + token-sharded LM head (no collective) ----
        xfT = layer_norm(xT, ln_param(gf_i, "gft"), ln_param(bf_i, "bft"), dtb)
        boutt = ln_param(bout_i, "boutt")
        for vg in range(VPAD // 1024):
            wvh = wqkv.tile([128, 8, 1024], dtb,
                            tag=("wqt", "wkt", "wvt")[vg % 3])
            nc.sync.dma_start(
                out=wvh[:],
                in_=wout_i.ap()[:, 1024 * vg:1024 * (vg + 1)].rearrange(
                    "(c p) m -> p c m", p=128))
            osb = act2.tile([128, 2048], dtb, tag="osb")
            for q8 in range(8):
                vm = 8 * vg + q8
                p_o = ps.tile([128, 1024], dtf, tag="bank")
                for c in range(8):
                    nc.tensor.matmul(
                        p_o[:, 0:256], wvh[:, c, 128 * q8:128 * (q8 + 1)],
                        cs(xfT[:], c),
                        start=(c == 0), stop=(c == 7))
                nc.vector.tensor_scalar(
                    out=osb[:, 256 * q8:256 * (q8 + 1)], in0=p_o[:, 0:256],
                    scalar1=boutt[:, vm:vm + 1],
                    scalar2=None, op0=OP.add)
            nc.sync.dma_start(
                out=out_o[:, 2048 * vg:2048 * (vg + 1)], in_=osb[:])


def _host_inputs(inputs):
    bf16 = ml_dtypes.bfloat16
    tokens = np.asarray(inputs["tokens"])
    emb = np.asarray(inputs["emb"], dtype=np.float32)
    pe = np.asarray(inputs["pe"], dtype=np.float32)
    Wq = np.asarray(inputs["Wq"], dtype=np.float32)
    Wk = np.asarray(inputs["Wk"], dtype=np.float32)
    Wv = np.asarray(inputs["Wv"], dtype=np.float32)
    Wo = np.asarray(inputs["Wo"], dtype=np.float32)
    bo = np.asarray(inputs["bo"], dtype=np.float32)
    g1 = np.asarray(inputs["g1"], dtype=np.float32)
    be1 = np.asarray(inputs["be1"], dtype=np.float32)
    g2 = np.asarray(inputs["g2"], dtype=np.float32)
    be2 = np.asarray(inputs["be2"], dtype=np.float32)
    W1 = np.asarray(inputs["W1"], dtype=np.float32)
    b1 = np.asarray(inputs["b1"], dtype=np.float32)
    W2 = np.asarray(inputs["W2"], dtype=np.float32)
    b2 = np.asarray(inputs["b2"], dtype=np.float32)
    gf = np.asarray(inputs["gf"], dtype=np.float32)
    bf = np.asarray(inputs["bf"], dtype=np.float32)
    Wout = np.asarray(inputs["Wout"], dtype=np.float32)
    bout = np.asarray(inputs["bout"], dtype=np.float32)

    x0 = emb[tokens] * math.sqrt(float(D)) + pe[:S][None]   # (B, S, D)
    xflat = np.ascontiguousarray(x0.reshape(T, D))

    padb = np.where(tokens == 0, np.float32(NEG), np.float32(0.0))
    padb = np.ascontiguousarray(padb.reshape(2, 8, 128).astype(np.float32))
    r = np.arange(128)
    causal = np.where(r[:, None] > r[None, :], np.float32(NEG),
                      np.float32(0.0)).astype(np.float32)

    def tchunks(a, n):   # [L, D'] -> [L, 128, n] feature-major chunks
        return np.ascontiguousarray(
            a.reshape(L, n, 128).transpose(0, 2, 1).astype(np.float32))

    g1t, be1t = tchunks(g1, 8), tchunks(be1, 8)
    g2t, be2t = tchunks(g2, 8), tchunks(be2, 8)
    bot, b2t = tchunks(bo, 8), tchunks(b2, 8)
    b1t = tchunks(b1, 32)
    gft = np.ascontiguousarray(gf.reshape(8, 128).T.astype(np.float32))
    bft = np.ascontiguousarray(bf.reshape(8, 128).T.astype(np.float32))

    wout_p = np.zeros((D, VPAD), dtype=np.float32)
    wout_p[:, :V] = Wout
    bout_p = np.zeros((VPAD,), dtype=np.float32)
    bout_p[:V] = bout

    wq_s = (Wq / math.sqrt(float(DK))).astype(np.float32)

    shared = dict(
        wq=np.ascontiguousarray(wq_s.astype(bf16)),
        wk=np.ascontiguousarray(Wk.astype(bf16)),
        wv=np.ascontiguousarray(Wv.astype(bf16)),
        wo=np.ascontiguousarray(Wo.astype(bf16)),
        w1=np.ascontiguousarray(W1.astype(bf16)),
        w2=np.ascontiguousarray(W2.astype(bf16)),
        wout=np.ascontiguousarray(wout_p.astype(bf16)),
        bout=np.ascontiguousarray(bout_p.reshape(256, 128).T.astype(np.float32)),
        g1=g1t, be1=be1t, g2=g2t, be2=be2t, bo=bot, b1=b1t, b2=b2t,
        gf=gft, bf=bft, causal=causal, padb=padb,
    )
    in_maps = []
    for c in range(NCORES):
        m = dict(shared)
        m["x0t"] = np.ascontiguousarray(
            xflat[TPC * c:TPC * (c + 1)].T)
        in_maps.append(m)
    return in_maps


def kernel(**inputs):
    from concourse import bass_utils

    if "nc" not in _CACHE:
        _CACHE["nc"] = _build()
    nc = _CACHE["nc"]
    in_maps = _host_inputs(inputs)
    res = bass_utils.run_bass_kernel_spmd(
        nc, in_maps, core_ids=list(range(NCORES)))
    full = np.empty((T, V), dtype=np.float32)
    for c in range(NCORES):
        lt = np.asarray(res.results[c]["logits_t"])  # [128, 65536] bf16
        # vocab v = 1024*vg + 128*q8 + p at [p, 2048*vg + 256*q8 + t]
        lv = lt.reshape(128, 32, 8, TPC).transpose(1, 2, 0, 3).reshape(
            VPAD, TPC)
        full[TPC * c:TPC * (c + 1), :] = lv[:V, :].T.astype(np.float32)
    return full.reshape(B, S, V)


if __name__ == "__main__":
    import sys
    sys.path.insert(0, "/root/problem")
    import reference
    inp = reference.setup_inputs()
    out = kernel(**{k: np.asarray(v) for k, v in inp.items()})
    print("kernel output", out.shape, out.dtype)
